# revision 26
# baseline (speedup 1.0000x reference)
"""Causal multi-head attention (B=4, S=2048, D=1024, H=16, Hd=64) on 8 TRN2
NeuronCores.

Sharding: tensor-parallel over heads. Core c owns heads [2c, 2c+1]:
  - Wq/Wk/Wv column-sharded (rows of the [out,in] weight): each core projects
    x -> qT/kT/vT [128, S] (2 heads x 64, head-dim-major).
  - Attention per (b, h) computed entirely on-core, scoresT layout
    [keys, queries] so softmax normalization folds into matmuls.
  - Wo row-sharded: each core emits a partial [B,S,D] output; host sums the
    8 partials.

v2 vs baseline:
  - Causal trim: the 4 diagonal key-strips of each query chunk compute only
    queries >= strip start (widths 512/384/256/128 instead of 4x512), cutting
    scores+AV matmul columns and exp volume ~15%. Strips are packed in pairs
    (0,3) and (1,2) into [128,2,512] PSUM tiles so every exp reads only
    written data at exact widths.
  - Single [128,128] lower-triangle mask applied to the leading 128 columns
    of each diagonal strip (one strided multiply per head/pair).
  - One-group score lookahead: scores of group g+1 are emitted before the
    exp-dependent AV matmuls of group g so the PE never idles on the ACT.
  - Normalize reads pav PSUM directly (no [65,TC] bounce copy).
  - Output projection folded into the qc loop for ALL batches (no lump
    between batches; ACT/PE stay fed).

Numerics: matmul operands in bf16 (fp32 PSUM accumulation), softmax without
max-subtraction (scores are bounded ~|10| for this data distribution), causal
mask applied post-exp as a {0,1} multiply.
"""

import os
import numpy as np
import ml_dtypes
from contextlib import ExitStack

import concourse.bass as bass
import concourse.tile as tile
from concourse import bacc, mybir
from concourse.bass_utils import run_bass_kernel_spmd
from concourse.masks import make_identity

F32 = mybir.dt.float32
BF16 = mybir.dt.bfloat16
NPBF16 = ml_dtypes.bfloat16

B, S, D = 4, 2048, 1024
H, HD = 16, 64
NCORES = 8
HPC = H // NCORES          # heads per core
DH = HPC * HD              # local head dim (128)
TC = 512                   # token chunk for projections / query chunk
KS = 128                   # key strip

last_exec_time_ns = None   # set by kernel() when BASS_TRACE=1


def emit(tc_ctx: tile.TileContext, ctx: ExitStack, aps: dict, b_count: int, seq: int):
    """Emit the per-core program. aps: xt [b,D,seq] bf16, wq/wk/wv [D,DH] bf16,
    wo [DH,D] bf16, mask [128, 128] bf16, out [b,seq,D] bf16."""
    nc = tc_ctx.nc
    tc = tc_ctx
    KC = D // 128            # contraction chunks for projections
    NTC = seq // TC          # token chunks
    NQC = seq // TC          # query chunks
    NKS = seq // KS          # key strips

    xt, wq, wk, wv, wo, mask, out = (
        aps["xt"], aps["wq"], aps["wk"], aps["wv"], aps["wo"], aps["mask"], aps["out"]
    )

    wpool = ctx.enter_context(tc.tile_pool(name="wpool", bufs=1))
    xpool = ctx.enter_context(tc.tile_pool(name="xpool", bufs=4))
    qkpool = ctx.enter_context(tc.tile_pool(name="qkpool", bufs=4))
    vpool = ctx.enter_context(tc.tile_pool(name="vpool", bufs=2))
    ppool = ctx.enter_context(tc.tile_pool(name="ppool", bufs=4))
    avpool = ctx.enter_context(tc.tile_pool(name="avpool", bufs=4))
    smalls = ctx.enter_context(tc.tile_pool(name="smalls", bufs=4))

    ps_scr = ctx.enter_context(tc.tile_pool(name="ps_scr", bufs=2, space="PSUM"))
    ps_p = ctx.enter_context(tc.tile_pool(name="ps_p", bufs=2, space="PSUM"))
    ps_av = ctx.enter_context(tc.tile_pool(name="ps_av", bufs=2, space="PSUM"))

    # --- constants / weights ---
    w_sb = {}
    for name, ap in (("wq", wq), ("wk", wk), ("wv", wv)):
        t = wpool.tile([128, KC, DH], BF16, tag=name, name=f"w_{name}")
        nc.sync.dma_start(out=t, in_=ap.rearrange("(kc p) m -> p kc m", p=128))
        w_sb[name] = t
    wo_sb = wpool.tile([128, D], BF16)
    nc.sync.dma_start(out=wo_sb, in_=wo)
    mask_sb = wpool.tile([128, 128], BF16)
    nc.sync.dma_start(out=mask_sb, in_=mask)

    ident_f = wpool.tile([128, 64], F32)
    make_identity(nc, ident_f[0:64, :])
    make_identity(nc, ident_f[64:128, :])
    ident = wpool.tile([128, 64], BF16)
    nc.vector.tensor_copy(ident, ident_f)

    ones_f = wpool.tile([128, 64], F32)
    nc.vector.memset(ones_f, 1.0)
    ones_r = wpool.tile([128, 64], BF16)
    nc.vector.tensor_copy(ones_r, ones_f)

    qTs, kTs, vexts, avTs = {}, {}, {}, {}

    def proj_fillers(b, tcc):
        """Projection work for one 512-token chunk of batch b, as a list of
        closures so it can be interleaved between attention groups."""
        def dmas():
            xt_src = xt[b].rearrange("(kh kc p) t -> p kh kc t", p=128, kh=2)
            for kh in range(2):  # two 4-chunk DMAs instead of eight 1-chunk
                t = xpool.tile([128, KC // 2, TC], BF16, tag="xt",
                               name=f"xt_{b}_{tcc}_{kh}", bufs=6)
                nc.sync.dma_start(out=t,
                                  in_=xt_src[:, kh, :, tcc * TC:(tcc + 1) * TC])
                xt_ts[(b, tcc, kh)] = t

        def wgroup(name):
            def go():
                dst = {"wq": qTs[b], "wk": kTs[b], "wv": vexts[(b, "vT")]}[name]
                ps = ps_scr.tile([128, TC], F32, tag="scr", name=f"ps_{name}")
                for kc in range(KC):
                    nc.tensor.matmul(ps, w_sb[name][:, kc, :],
                                     xt_ts[(b, tcc, kc // 4)][:, kc % 4, :],
                                     start=(kc == 0), stop=(kc == KC - 1))
                nc.vector.tensor_copy(dst[:, tcc * TC:(tcc + 1) * TC], ps)
            return go

        def vtrans(h):
            def go():
                vext = vexts[b]
                vT = vexts[(b, "vT")]
                tr4 = ps_scr.tile([128, 4, 64], BF16, tag="scr", name="tr4")
                for i in range(4):
                    ks = tcc * 4 + i
                    nc.tensor.transpose(
                        tr4[:, i, :], vT[h * 64:(h + 1) * 64, ks * 128:(ks + 1) * 128],
                        ident[h * 64:(h + 1) * 64, :])
                nc.vector.tensor_copy(vext[:, h, tcc * 4:(tcc + 1) * 4, 0:64], tr4)
                nc.vector.tensor_copy(vext[:, h, tcc * 4:(tcc + 1) * 4, 64:65],
                                      ones_r[:, 0:1].to_broadcast([128, 4, 1]))
            return go

        return [dmas, wgroup("wq"), wgroup("wk"), wgroup("wv"),
                vtrans(0), vtrans(1)]

    def alloc_batch(b):
        qTs[b] = qkpool.tile([128, seq], BF16, tag="qT", name=f"qT{b}")
        kTs[b] = qkpool.tile([128, seq], BF16, tag="kT", name=f"kT{b}")
        vexts[(b, "vT")] = vpool.tile([128, seq], BF16, tag="vT", name=f"vT{b}")
        vexts[b] = vpool.tile([128, HPC, NKS, 65], BF16, tag="vext",
                              name=f"vext{b}", bufs=4)

    def emit_attn_qc(b, qc, fillers):
        """One query-chunk of attention for batch b, causal-trimmed.

        Strip groups (each a [128,2,512] PSUM pair, one tile per head):
          - 2*qc pairs of full strips (widths 512/512)
          - diagonal pair A: strips 4qc+0 (w=512, q_off=0) and 4qc+3
            (w=128, q_off=384) -> one exp over contiguous [128, 640]
          - diagonal pair B: strips 4qc+1 (w=384, q_off=128) and 4qc+2
            (w=256, q_off=256) -> two exact-width exps
        Scores of group g+1 are emitted before AV of group g (PE lookahead);
        one filler (independent proj/outproj PE work) is woven between them
        so the PE never stalls on the exp.
        """
        qT, kT, vext = qTs[b], kTs[b], vexts[b]
        avT = avTs[b]
        d0 = 4 * qc
        # one group per key strip; BOTH heads share a [128, 2(head), 512]
        # PSUM tile, so ps_p's bufs=2 gives a true one-group lookahead.
        # group := (st, q_off, width, diag?)
        groups = [(st, 0, TC, False) for st in range(d0)]
        for i in range(4):
            groups.append((d0 + i, 128 * i, TC - 128 * i, True))
        last_av_st = d0 + 3

        pav = {h: ps_av.tile([65, TC], F32, tag="av", name=f"pav{h}")
               for h in range(HPC)}

        def emit_scores(grp):
            st, qo, w, diag = grp
            pp = ps_p.tile([128, 2, TC], F32, tag="pp", name="pp")
            for h in range(HPC):
                nc.tensor.matmul(
                    pp[:, h, 0:w],
                    kT[h * 64:(h + 1) * 64, st * KS:(st + 1) * KS],
                    qT[h * 64:(h + 1) * 64, qc * TC + qo:qc * TC + qo + w],
                    start=True, stop=True)
            return pp

        def emit_expmaskav(grp, pp):
            st, qo, w, diag = grp
            p_sb = ppool.tile([128, 2, TC], BF16, tag="p", name="p_sb")
            if w == TC:
                nc.scalar.activation(p_sb.rearrange("p a b -> p (a b)"),
                                     pp.rearrange("p a b -> p (a b)"),
                                     mybir.ActivationFunctionType.Exp)
            else:
                nc.scalar.activation(p_sb[:, :, 0:w], pp[:, :, 0:w],
                                     mybir.ActivationFunctionType.Exp)
            if diag:  # triangle mask on the leading 128 cols of both heads
                nc.vector.tensor_mul(
                    p_sb[:, :, 0:128], p_sb[:, :, 0:128],
                    mask_sb.rearrange("p (o m) -> p o m", o=1)
                    .to_broadcast([128, 2, 128]))
            for h in range(HPC):
                nc.tensor.matmul(pav[h][:, qo:qo + w], vext[:, h, st, :],
                                 p_sb[:, h, 0:w],
                                 start=(st == 0), stop=(st == last_av_st))

        fi = 0
        def fill(n):
            nonlocal fi
            for _ in range(n):
                if fi < len(fillers):
                    fillers[fi]()
                    fi += 1

        prev = None
        ngaps = len(groups)
        for gi, grp in enumerate(groups):
            pp = emit_scores(grp)
            # distribute fillers evenly across the group gaps
            want = -((len(fillers) - fi) // -(ngaps - gi))
            fill(want)
            if prev is not None:
                emit_expmaskav(prev[0], prev[1])
            prev = (grp, pp)
        emit_expmaskav(prev[0], prev[1])

        for h in range(HPC):
            z0 = smalls.tile([1, TC], F32, tag="z0")
            nc.vector.tensor_copy(z0, pav[h][64:65, :])
            zbb = smalls.tile([64, TC], F32, tag="zbb")
            nc.gpsimd.partition_broadcast(zbb, z0)
            rz = smalls.tile([64, TC], F32, tag="rz")
            nc.vector.reciprocal_approx_fast(rz, zbb)
            with nc.allow_low_precision(reason="attn weights tolerate bf16"):
                nc.vector.tensor_mul(avT[h * 64:(h + 1) * 64, qc * TC:(qc + 1) * TC],
                                     pav[h][0:64, :], rz)
        # output projection of THIS chunk, on the pp banks (idle once the
        # last score group is consumed) -- keeps the scr banks free for proj
        for t4 in range(TC // 128):
            t16 = qc * (TC // 128) + t4
            po2 = ps_p.tile([128, 2, TC], F32, tag="pp", name="po2")
            o_sb = smalls.tile([128, 2, TC], BF16, tag="o")
            for n2 in range(D // TC):
                nc.tensor.matmul(po2[:, n2, :], avT[:, t16 * 128:(t16 + 1) * 128],
                                 wo_sb[:, n2 * TC:(n2 + 1) * TC],
                                 start=True, stop=True)
                if (t4 + n2) % 2 == 0:
                    nc.vector.tensor_copy(o_sb[:, n2, :], po2[:, n2, :])
                else:
                    nc.scalar.copy(o_sb[:, n2, :], po2[:, n2, :])
            nc.sync.dma_start(
                out=out[b, t16 * 128:(t16 + 1) * 128, :],
                in_=o_sb.rearrange("p a b -> p (a b)"))
            fill(1)
        fill(len(fillers) - fi)

    def outproj_fillers(b, qc, split_heads=False):
        """Output projection for the 4 token-128 blocks of chunk qc of batch
        b, as filler closures (one per token-128 block). split_heads breaks
        the contraction into the two head halves so the first matmul can
        start as soon as head 0's normalize lands (used for the final chunk
        to shorten the kernel tail)."""
        def piece(t4):
            def go():
                avT = avTs[b]
                t16 = qc * (TC // 128) + t4
                o_sb = smalls.tile([128, 2, TC], BF16, tag="o")
                for n2 in range(D // TC):
                    po = ps_scr.tile([128, TC], F32, tag="scr", name="po")
                    nc.tensor.matmul(po, avT[:, t16 * 128:(t16 + 1) * 128],
                                     wo_sb[:, n2 * TC:(n2 + 1) * TC],
                                     start=True, stop=True)
                    if (t4 + n2) % 2 == 0:
                        nc.vector.tensor_copy(o_sb[:, n2, :], po)
                    else:
                        nc.scalar.copy(o_sb[:, n2, :], po)
                nc.sync.dma_start(
                    out=out[b, t16 * 128:(t16 + 1) * 128, :],
                    in_=o_sb.rearrange("p a b -> p (a b)"))
            return go
        return [piece(t4) for t4 in range(TC // 128)]

    # Self-weaving pipeline: attention(b, qc) interleaves, at group
    # granularity, the projection of chunk qc+1 of the SAME batch (chunk 0 of
    # b+1 at the last qc) plus the output projection of the previous chunk.
    # Every qc of every batch -- including the last -- has independent PE
    # filler, so the tensor engine never drains while the ACT runs exp.
    xt_ts = {}
    alloc_batch(0)
    avTs[0] = avpool.tile([128, seq], BF16, tag="avT", name="avT0")
    for f in proj_fillers(0, 0):
        f()
    for b in range(b_count):
        if b + 1 < b_count:
            alloc_batch(b + 1)
            avTs[b + 1] = avpool.tile([128, seq], BF16, tag="avT",
                                      name=f"avT{b + 1}")
        for qc in range(NQC):
            fillers = []
            if qc + 1 < NQC:
                fillers = proj_fillers(b, qc + 1)
            elif b + 1 < b_count:
                fillers = proj_fillers(b + 1, 0)
            emit_attn_qc(b, qc, fillers)


def host_inputs(x, Wq, Wk, Wv, Wo, core, xt_bf=None):
    """Build the per-core input map."""
    hs = slice(core * DH, (core + 1) * DH)
    if xt_bf is None:
        xt_bf = np.ascontiguousarray(np.transpose(x, (0, 2, 1))).astype(NPBF16)
    wq = np.ascontiguousarray((Wq[hs, :] * np.float32(1.0 / np.sqrt(HD))).T).astype(NPBF16)
    wk = np.ascontiguousarray(Wk[hs, :].T).astype(NPBF16)
    wv = np.ascontiguousarray(Wv[hs, :].T).astype(NPBF16)
    wo = np.ascontiguousarray(Wo[:, hs].T).astype(NPBF16)
    # [128,128] lower-triangle (keys on partitions): mask[k, q] = (q >= k)
    mask = (np.arange(128)[None, :] >= np.arange(128)[:, None]).astype(NPBF16)
    return {"xt": xt_bf, "wq": wq, "wk": wk, "wv": wv, "wo": wo, "mask": mask}


def build_program(b_count=B, seq=S):
    nc = bacc.Bacc("TRN2", target_bir_lowering=False, debug=False,
                   num_devices=NCORES)
    aps = {
        "xt": nc.dram_tensor("xt", [b_count, D, seq], BF16, kind="ExternalInput").ap(),
        "wq": nc.dram_tensor("wq", [D, DH], BF16, kind="ExternalInput").ap(),
        "wk": nc.dram_tensor("wk", [D, DH], BF16, kind="ExternalInput").ap(),
        "wv": nc.dram_tensor("wv", [D, DH], BF16, kind="ExternalInput").ap(),
        "wo": nc.dram_tensor("wo", [DH, D], BF16, kind="ExternalInput").ap(),
        "mask": nc.dram_tensor("mask", [128, 128], BF16, kind="ExternalInput").ap(),
        "out": nc.dram_tensor("out", [b_count, seq, D], BF16, kind="ExternalOutput").ap(),
    }
    with tile.TileContext(nc) as tcx:
        with ExitStack() as ctx:
            emit(tcx, ctx, aps, b_count, seq)
    nc.finalize()
    return nc


def _ensure_ntff_hook():
    """Register the ctypes NTFF profile hook when the image lacks
    antenv.axon_hooks (needed only for trace=True)."""
    import sys, types
    try:
        import antenv.axon_hooks  # noqa: F401
        return
    except ImportError:
        pass
    try:
        import antenv
        from trn_agent_boot.trn_boot import _ntff_profile_via_ctypes
        hook = _ntff_profile_via_ctypes("/opt/axon/libaxon_pjrt.so")
        mod = types.ModuleType("antenv.axon_hooks")
        mod.get_axon_ntff_profile_hook = lambda: hook
        mod.set_axon_ntff_profile_hook = lambda h: None
        sys.modules["antenv.axon_hooks"] = mod
        antenv.axon_hooks = mod
    except Exception:
        pass


def kernel(x, Wq, Wk, Wv, Wo):
    global last_exec_time_ns
    x = np.asarray(x, dtype=np.float32)
    Wq = np.asarray(Wq, dtype=np.float32)
    Wk = np.asarray(Wk, dtype=np.float32)
    Wv = np.asarray(Wv, dtype=np.float32)
    Wo = np.asarray(Wo, dtype=np.float32)

    nc = build_program(B, S)
    xt_bf = np.ascontiguousarray(np.transpose(x, (0, 2, 1))).astype(NPBF16)
    in_maps = [host_inputs(x, Wq, Wk, Wv, Wo, c, xt_bf=xt_bf) for c in range(NCORES)]
    trace = bool(os.environ.get("BASS_TRACE"))
    if trace:
        _ensure_ntff_hook()
    res = run_bass_kernel_spmd(nc, in_maps, list(range(NCORES)), trace=trace)
    last_exec_time_ns = res.exec_time_ns
    parts = [res.results[c]["out"] for c in range(NCORES)]
    acc = parts[0].astype(np.float32)
    for p in parts[1:]:
        acc = acc + p
    return acc


# revision 28
# speedup vs baseline: 1.2606x; 1.2606x over previous
"""Causal multi-head attention (B=4, S=2048, D=1024, H=16, Hd=64) on 8 TRN2
NeuronCores.

Sharding: tensor-parallel over heads. Core c owns heads [2c, 2c+1]:
  - Wq/Wk/Wv column-sharded (rows of the [out,in] weight): each core projects
    x -> qT/kT/vT [128, S] (2 heads x 64, head-dim-major).
  - Attention per (b, h) computed entirely on-core, scoresT layout
    [keys, queries] so softmax normalization folds into matmuls.
  - Wo row-sharded: each core emits a partial [B,S,D] output; host sums the
    8 partials.

v2 vs baseline:
  - Causal trim: the 4 diagonal key-strips of each query chunk compute only
    queries >= strip start (widths 512/384/256/128 instead of 4x512), cutting
    scores+AV matmul columns and exp volume ~15%. Strips are packed in pairs
    (0,3) and (1,2) into [128,2,512] PSUM tiles so every exp reads only
    written data at exact widths.
  - Single [128,128] lower-triangle mask applied to the leading 128 columns
    of each diagonal strip (one strided multiply per head/pair).
  - One-group score lookahead: scores of group g+1 are emitted before the
    exp-dependent AV matmuls of group g so the PE never idles on the ACT.
  - Normalize reads pav PSUM directly (no [65,TC] bounce copy).
  - Output projection folded into the qc loop for ALL batches (no lump
    between batches; ACT/PE stay fed).

Numerics: matmul operands in bf16 (fp32 PSUM accumulation), softmax without
max-subtraction (scores are bounded ~|10| for this data distribution), causal
mask applied post-exp as a {0,1} multiply.
"""

import os
import numpy as np
import ml_dtypes
from contextlib import ExitStack

import concourse.bass as bass
import concourse.tile as tile
from concourse import bacc, mybir
from concourse.bass_utils import run_bass_kernel_spmd
from concourse.masks import make_identity

F32 = mybir.dt.float32
BF16 = mybir.dt.bfloat16
NPBF16 = ml_dtypes.bfloat16

B, S, D = 4, 2048, 1024
H, HD = 16, 64
NCORES = 8
HPC = H // NCORES          # heads per core
DH = HPC * HD              # local head dim (128)
TC = 512                   # token chunk for projections / query chunk
KS = 128                   # key strip

last_exec_time_ns = None   # set by kernel() when BASS_TRACE=1


def emit(tc_ctx: tile.TileContext, ctx: ExitStack, aps: dict, b_count: int, seq: int):
    """Emit the per-core program. aps: xt [b,D,seq] bf16, wq/wk/wv [D,DH] bf16,
    wo [DH,D] bf16, mask [128, 128] bf16, out [b,seq,D] bf16."""
    nc = tc_ctx.nc
    tc = tc_ctx
    KC = D // 128            # contraction chunks for projections
    NTC = seq // TC          # token chunks
    NQC = seq // TC          # query chunks
    NKS = seq // KS          # key strips

    xt, wq, wk, wv, wo, mask, out = (
        aps["xt"], aps["wq"], aps["wk"], aps["wv"], aps["wo"], aps["mask"], aps["out"]
    )

    wpool = ctx.enter_context(tc.tile_pool(name="wpool", bufs=1))
    xpool = ctx.enter_context(tc.tile_pool(name="xpool", bufs=4))
    qkpool = ctx.enter_context(tc.tile_pool(name="qkpool", bufs=4))
    vpool = ctx.enter_context(tc.tile_pool(name="vpool", bufs=2))
    ppool = ctx.enter_context(tc.tile_pool(name="ppool", bufs=4))
    avpool = ctx.enter_context(tc.tile_pool(name="avpool", bufs=4))
    smalls = ctx.enter_context(tc.tile_pool(name="smalls", bufs=4))

    ps_scr = ctx.enter_context(tc.tile_pool(name="ps_scr", bufs=2, space="PSUM"))
    ps_p = ctx.enter_context(tc.tile_pool(name="ps_p", bufs=2, space="PSUM"))
    ps_av = ctx.enter_context(tc.tile_pool(name="ps_av", bufs=2, space="PSUM"))

    # --- constants / weights ---
    w_sb = {}
    for name, ap in (("wq", wq), ("wk", wk), ("wv", wv)):
        t = wpool.tile([128, KC, DH], BF16, tag=name, name=f"w_{name}")
        nc.sync.dma_start(out=t, in_=ap.rearrange("(kc p) m -> p kc m", p=128))
        w_sb[name] = t
    wo_sb = wpool.tile([128, D], BF16)
    nc.sync.dma_start(out=wo_sb, in_=wo)
    mask_sb = wpool.tile([128, 128], BF16)
    nc.sync.dma_start(out=mask_sb, in_=mask)

    ident_f = wpool.tile([128, 64], F32)
    make_identity(nc, ident_f[0:64, :])
    make_identity(nc, ident_f[64:128, :])
    ident = wpool.tile([128, 64], BF16)
    nc.vector.tensor_copy(ident, ident_f)

    ones_f = wpool.tile([128, 64], F32)
    nc.vector.memset(ones_f, 1.0)
    ones_r = wpool.tile([128, 64], BF16)
    nc.vector.tensor_copy(ones_r, ones_f)

    qTs, kTs, vexts, avTs = {}, {}, {}, {}

    def proj_fillers(b, tcc):
        """Projection work for one 512-token chunk of batch b, as a list of
        closures so it can be interleaved between attention groups."""
        def dmas():
            xt_src = xt[b].rearrange("(kh kc p) t -> p kh kc t", p=128, kh=2)
            for kh in range(2):  # two 4-chunk DMAs instead of eight 1-chunk
                t = xpool.tile([128, KC // 2, TC], BF16, tag="xt",
                               name=f"xt_{b}_{tcc}_{kh}", bufs=6)
                nc.sync.dma_start(out=t,
                                  in_=xt_src[:, kh, :, tcc * TC:(tcc + 1) * TC])
                xt_ts[(b, tcc, kh)] = t

        def wgroup(name):
            def go():
                dst = {"wq": qTs[b], "wk": kTs[b], "wv": vexts[(b, "vT")]}[name]
                ps = ps_scr.tile([128, TC], F32, tag="scr", name=f"ps_{name}")
                for kc in range(KC):
                    nc.tensor.matmul(ps, w_sb[name][:, kc, :],
                                     xt_ts[(b, tcc, kc // 4)][:, kc % 4, :],
                                     start=(kc == 0), stop=(kc == KC - 1))
                nc.vector.tensor_copy(dst[:, tcc * TC:(tcc + 1) * TC], ps)
            return go

        def vtrans(h):
            def go():
                vext = vexts[b]
                vT = vexts[(b, "vT")]
                tr4 = ps_scr.tile([128, 4, 64], BF16, tag="scr", name="tr4")
                for i in range(4):
                    ks = tcc * 4 + i
                    nc.tensor.transpose(
                        tr4[:, i, :], vT[h * 64:(h + 1) * 64, ks * 128:(ks + 1) * 128],
                        ident[h * 64:(h + 1) * 64, :])
                nc.vector.tensor_copy(vext[:, h, tcc * 4:(tcc + 1) * 4, 0:64], tr4)
                nc.vector.tensor_copy(vext[:, h, tcc * 4:(tcc + 1) * 4, 64:65],
                                      ones_r[:, 0:1].to_broadcast([128, 4, 1]))
            return go

        return [dmas, wgroup("wq"), wgroup("wk"), wgroup("wv"),
                vtrans(0), vtrans(1)]

    def alloc_batch(b):
        qTs[b] = qkpool.tile([128, seq], BF16, tag="qT", name=f"qT{b}")
        kTs[b] = qkpool.tile([128, seq], BF16, tag="kT", name=f"kT{b}")
        vexts[(b, "vT")] = vpool.tile([128, seq], BF16, tag="vT", name=f"vT{b}")
        vexts[b] = vpool.tile([128, HPC, NKS, 65], BF16, tag="vext",
                              name=f"vext{b}", bufs=4)

    def emit_attn_qc(b, qc, fillers):
        """One query-chunk of attention for batch b, causal-trimmed.

        Strip groups (each a [128,2,512] PSUM pair, one tile per head):
          - 2*qc pairs of full strips (widths 512/512)
          - diagonal pair A: strips 4qc+0 (w=512, q_off=0) and 4qc+3
            (w=128, q_off=384) -> one exp over contiguous [128, 640]
          - diagonal pair B: strips 4qc+1 (w=384, q_off=128) and 4qc+2
            (w=256, q_off=256) -> two exact-width exps
        Scores of group g+1 are emitted before AV of group g (PE lookahead);
        one filler (independent proj/outproj PE work) is woven between them
        so the PE never stalls on the exp.
        """
        qT, kT, vext = qTs[b], kTs[b], vexts[b]
        avT = avTs[b]
        d0 = 4 * qc
        # one group per key strip; BOTH heads share a [128, 2(head), 512]
        # PSUM tile, so ps_p's bufs=2 gives a true one-group lookahead.
        # group := (st, q_off, width, diag?)
        groups = [(st, 0, TC, False) for st in range(d0)]
        for i in range(4):
            groups.append((d0 + i, 128 * i, TC - 128 * i, True))
        last_av_st = d0 + 3

        pav = {h: ps_av.tile([65, TC], F32, tag="av", name=f"pav{h}")
               for h in range(HPC)}

        def emit_scores(grp):
            st, qo, w, diag = grp
            pp = ps_p.tile([128, 2, TC], F32, tag="pp", name="pp")
            for h in range(HPC):
                nc.tensor.matmul(
                    pp[:, h, 0:w],
                    kT[h * 64:(h + 1) * 64, st * KS:(st + 1) * KS],
                    qT[h * 64:(h + 1) * 64, qc * TC + qo:qc * TC + qo + w],
                    start=True, stop=True)
            return pp

        def emit_expmaskav(grp, pp):
            st, qo, w, diag = grp
            p_sb = ppool.tile([128, 2, TC], BF16, tag="p", name="p_sb")
            if w == TC:
                nc.scalar.activation(p_sb.rearrange("p a b -> p (a b)"),
                                     pp.rearrange("p a b -> p (a b)"),
                                     mybir.ActivationFunctionType.Exp)
            else:
                nc.scalar.activation(p_sb[:, :, 0:w], pp[:, :, 0:w],
                                     mybir.ActivationFunctionType.Exp)
            if diag:  # triangle mask on the leading 128 cols of both heads
                nc.vector.tensor_mul(
                    p_sb[:, :, 0:128], p_sb[:, :, 0:128],
                    mask_sb.rearrange("p (o m) -> p o m", o=1)
                    .to_broadcast([128, 2, 128]))
            for h in range(HPC):
                nc.tensor.matmul(pav[h][:, qo:qo + w], vext[:, h, st, :],
                                 p_sb[:, h, 0:w],
                                 start=(st == 0), stop=(st == last_av_st))

        fi = 0
        def fill(n):
            nonlocal fi
            for _ in range(n):
                if fi < len(fillers):
                    fillers[fi]()
                    fi += 1

        prev = None
        ngaps = len(groups)
        for gi, grp in enumerate(groups):
            pp = emit_scores(grp)
            # distribute fillers evenly across the group gaps
            want = -((len(fillers) - fi) // -(ngaps - gi))
            fill(want)
            if prev is not None:
                emit_expmaskav(prev[0], prev[1])
            prev = (grp, pp)
        emit_expmaskav(prev[0], prev[1])

        for h in range(HPC):
            z0 = smalls.tile([1, TC], F32, tag="z0")
            nc.vector.tensor_copy(z0, pav[h][64:65, :])
            zbb = smalls.tile([64, TC], F32, tag="zbb")
            nc.gpsimd.partition_broadcast(zbb, z0)
            rz = smalls.tile([64, TC], F32, tag="rz")
            nc.vector.reciprocal_approx_fast(rz, zbb)
            with nc.allow_low_precision(reason="attn weights tolerate bf16"):
                nc.vector.tensor_mul(avT[h * 64:(h + 1) * 64, qc * TC:(qc + 1) * TC],
                                     pav[h][0:64, :], rz)
        fill(len(fillers) - fi)

    def outproj_fillers(b, qc, split_heads=False):
        """Output projection for the 4 token-128 blocks of chunk qc of batch
        b, as filler closures (one per token-128 block). split_heads breaks
        the contraction into the two head halves so the first matmul can
        start as soon as head 0's normalize lands (used for the final chunk
        to shorten the kernel tail)."""
        def piece(t4):
            def go():
                avT = avTs[b]
                t16 = qc * (TC // 128) + t4
                o_sb = smalls.tile([128, 2, TC], BF16, tag="o")
                for n2 in range(D // TC):
                    po = ps_scr.tile([128, TC], F32, tag="scr", name="po")
                    nc.tensor.matmul(po, avT[:, t16 * 128:(t16 + 1) * 128],
                                     wo_sb[:, n2 * TC:(n2 + 1) * TC],
                                     start=True, stop=True)
                    if (t4 + n2) % 2 == 0:
                        nc.vector.tensor_copy(o_sb[:, n2, :], po)
                    else:
                        nc.scalar.copy(o_sb[:, n2, :], po)
                nc.sync.dma_start(
                    out=out[b, t16 * 128:(t16 + 1) * 128, :],
                    in_=o_sb.rearrange("p a b -> p (a b)"))
            return go
        return [piece(t4) for t4 in range(TC // 128)]

    # Self-weaving pipeline: attention(b, qc) interleaves, at group
    # granularity, the projection of chunk qc+1 of the SAME batch (chunk 0 of
    # b+1 at the last qc) plus the output projection of the previous chunk.
    # Every qc of every batch -- including the last -- has independent PE
    # filler, so the tensor engine never drains while the ACT runs exp.
    xt_ts = {}
    alloc_batch(0)
    avTs[0] = avpool.tile([128, seq], BF16, tag="avT", name="avT0")
    for f in proj_fillers(0, 0):
        f()
    for b in range(b_count):
        if b + 1 < b_count:
            alloc_batch(b + 1)
            avTs[b + 1] = avpool.tile([128, seq], BF16, tag="avT",
                                      name=f"avT{b + 1}")
        for qc in range(NQC):
            pf, of = [], []
            if qc + 1 < NQC:
                pf = proj_fillers(b, qc + 1)
            elif b + 1 < b_count:
                pf = proj_fillers(b + 1, 0)
            if qc > 0:
                of = outproj_fillers(b, qc - 1)
            elif b > 0:
                of = outproj_fillers(b - 1, NQC - 1)
            emit_attn_qc(b, qc, pf + of)
    for f in outproj_fillers(b_count - 1, NQC - 1):
        f()


def host_inputs(x, Wq, Wk, Wv, Wo, core, xt_bf=None):
    """Build the per-core input map."""
    hs = slice(core * DH, (core + 1) * DH)
    if xt_bf is None:
        xt_bf = np.ascontiguousarray(np.transpose(x, (0, 2, 1))).astype(NPBF16)
    wq = np.ascontiguousarray((Wq[hs, :] * np.float32(1.0 / np.sqrt(HD))).T).astype(NPBF16)
    wk = np.ascontiguousarray(Wk[hs, :].T).astype(NPBF16)
    wv = np.ascontiguousarray(Wv[hs, :].T).astype(NPBF16)
    wo = np.ascontiguousarray(Wo[:, hs].T).astype(NPBF16)
    # [128,128] lower-triangle (keys on partitions): mask[k, q] = (q >= k)
    mask = (np.arange(128)[None, :] >= np.arange(128)[:, None]).astype(NPBF16)
    return {"xt": xt_bf, "wq": wq, "wk": wk, "wv": wv, "wo": wo, "mask": mask}


def build_program(b_count=B, seq=S):
    nc = bacc.Bacc("TRN2", target_bir_lowering=False, debug=False,
                   num_devices=NCORES)
    aps = {
        "xt": nc.dram_tensor("xt", [b_count, D, seq], BF16, kind="ExternalInput").ap(),
        "wq": nc.dram_tensor("wq", [D, DH], BF16, kind="ExternalInput").ap(),
        "wk": nc.dram_tensor("wk", [D, DH], BF16, kind="ExternalInput").ap(),
        "wv": nc.dram_tensor("wv", [D, DH], BF16, kind="ExternalInput").ap(),
        "wo": nc.dram_tensor("wo", [DH, D], BF16, kind="ExternalInput").ap(),
        "mask": nc.dram_tensor("mask", [128, 128], BF16, kind="ExternalInput").ap(),
        "out": nc.dram_tensor("out", [b_count, seq, D], BF16, kind="ExternalOutput").ap(),
    }
    with tile.TileContext(nc) as tcx:
        with ExitStack() as ctx:
            emit(tcx, ctx, aps, b_count, seq)
    nc.finalize()
    return nc


def _ensure_ntff_hook():
    """Register the ctypes NTFF profile hook when the image lacks
    antenv.axon_hooks (needed only for trace=True)."""
    import sys, types
    try:
        import antenv.axon_hooks  # noqa: F401
        return
    except ImportError:
        pass
    try:
        import antenv
        from trn_agent_boot.trn_boot import _ntff_profile_via_ctypes
        hook = _ntff_profile_via_ctypes("/opt/axon/libaxon_pjrt.so")
        mod = types.ModuleType("antenv.axon_hooks")
        mod.get_axon_ntff_profile_hook = lambda: hook
        mod.set_axon_ntff_profile_hook = lambda h: None
        sys.modules["antenv.axon_hooks"] = mod
        antenv.axon_hooks = mod
    except Exception:
        pass


def kernel(x, Wq, Wk, Wv, Wo):
    global last_exec_time_ns
    x = np.asarray(x, dtype=np.float32)
    Wq = np.asarray(Wq, dtype=np.float32)
    Wk = np.asarray(Wk, dtype=np.float32)
    Wv = np.asarray(Wv, dtype=np.float32)
    Wo = np.asarray(Wo, dtype=np.float32)

    nc = build_program(B, S)
    xt_bf = np.ascontiguousarray(np.transpose(x, (0, 2, 1))).astype(NPBF16)
    in_maps = [host_inputs(x, Wq, Wk, Wv, Wo, c, xt_bf=xt_bf) for c in range(NCORES)]
    trace = bool(os.environ.get("BASS_TRACE"))
    if trace:
        _ensure_ntff_hook()
    res = run_bass_kernel_spmd(nc, in_maps, list(range(NCORES)), trace=trace)
    last_exec_time_ns = res.exec_time_ns
    parts = [res.results[c]["out"] for c in range(NCORES)]
    acc = parts[0].astype(np.float32)
    for p in parts[1:]:
        acc = acc + p
    return acc
